# revision 1
# baseline (speedup 1.0000x reference)
"""MoE block (D=1024, H=4096, E=8, top-2) on 8 Trainium2 NeuronCores.

Strategy: expert-parallel with a sharded router.
Core r owns expert r (W1[r]/b1[r]/W2[r]/b2[r] shipped pre-cast to bf16) and
routes only its own 512-token shard of x:
  1. loads its x-slice, PE-transposes it, computes router logits [tok, E] in
     fp32, does the top-2 threshold softmax, and encodes per-expert
     (index, weight) streams in a 16-partition-wrapped layout (built with
     two PE transposes, no elementwise DMA),
  2. AllToAll ships each expert's stream to its owner core (32 KB), then the
     GPSIMD sparse_gather compacts the <=1152 selected tokens; index/weight
     tables are cleaned and spread fully on-chip,
  3. gathers the selected token rows from x with indirect DMA and
     PE-transposes them into [D-part, slot] bf16 layout,
  4. runs the expert FFN in bf16 (fp32 accumulate): hT = gelu(W1^T xc^T +
     b1); mm2 is split into two D-halves, each half is routing-weight
     scaled, scatter-written (indirect DMA; rows are unique per core, pads
     go to a trash row) into a zero-filled [T+128, 512] bf16 partial, and
     ReduceScattered over the 8 cores -- the first RS overlaps the second
     half's matmuls,
  5. core r returns token rows [512*r : 512*(r+1)] as fp32.
"""

import os
import sys
import numpy as np
import ml_dtypes

sys.path.insert(0, "/opt/trn_rl_repo")

import concourse.bass as bass            # noqa: E402
import concourse.mybir as mybir          # noqa: E402
import concourse.tile as tile            # noqa: E402
from concourse import bacc               # noqa: E402
from concourse import bass_utils         # noqa: E402
from concourse import library_config      # noqa: E402

T, D, H, E = 4096, 1024, 4096, 8
N_CORES = 8
MPAD = 1152
NCOLS = MPAD // 128          # 9
NIDX = MPAD // 16            # 72
SHARD = T // N_CORES         # 512
JT = SHARD // 128            # 4

f32 = mybir.dt.float32
bf16 = mybir.dt.bfloat16
i32 = mybir.dt.int32
i16 = mybir.dt.int16
u32 = mybir.dt.uint32

_kernel_cache = {}


def _build(has_br: bool, has_b2: bool, reps: int = 1):
    nc = bacc.Bacc("TRN2", target_bir_lowering=False, debug=False,
                   num_devices=N_CORES)
    x = nc.dram_tensor("x", [T, D], f32, kind="ExternalInput")
    xsliceT = nc.dram_tensor("xsliceT", [128, 8 * SHARD], f32,
                             kind="ExternalInput")
    w1a = nc.dram_tensor("w1a", [32, 128, 8 * 128], bf16,
                         kind="ExternalInput")
    b1s = nc.dram_tensor("b1s", [H], f32, kind="ExternalInput")
    w2a = nc.dram_tensor("w2a", [2, 32, 128, 512], bf16,
                         kind="ExternalInput")
    b2s = nc.dram_tensor("b2s", [D], f32, kind="ExternalInput")
    wr = nc.dram_tensor("wr", [D, E], f32, kind="ExternalInput")
    br = nc.dram_tensor("br", [E], f32, kind="ExternalInput")
    identc = nc.dram_tensor("identc", [128, 128], f32, kind="ExternalInput")
    iota4 = nc.dram_tensor("iota4", [128, JT], f32, kind="ExternalInput")
    slotio = nc.dram_tensor("slotio", [16, 256], f32, kind="ExternalInput")
    onesrow = nc.dram_tensor("onesrow", [1, 128], f32, kind="ExternalInput")
    out_shard = nc.dram_tensor("out_shard", [SHARD, D], f32,
                               kind="ExternalOutput")

    with tile.TileContext(nc) as tc:
        with tc.tile_pool(name="persist", bufs=1) as persist, \
             tc.tile_pool(name="dram", bufs=1, space="DRAM") as dram:

            ident = persist.tile([128, 128], f32)
            nc.sync.dma_start(ident[:], identc[:])
            wr_sb = persist.tile([128, 8, E], f32)
            nc.sync.dma_start(wr_sb[:], wr[:].rearrange("(o p) e -> p o e", p=128))
            b1_sb = persist.tile([128, 32], f32)
            nc.sync.dma_start(b1_sb[:], b1s[:].rearrange("(o p) -> p o", p=128))
            ones_sb = persist.tile([1, 128], f32)
            nc.sync.dma_start(ones_sb[:], onesrow[:])
            iota_sb = persist.tile([128, JT], f32)
            nc.sync.dma_start(iota_sb[:], iota4[:])
            slot_sb = persist.tile([16, 256], f32)
            nc.sync.dma_start(slot_sb[:], slotio[:])
            if has_br:
                br_row = persist.tile([1, E], f32)
                nc.sync.dma_start(br_row[:], br[None, :])

            lib_sg = nc.gpsimd.load_library(library_config.sparse_gather)

            partial = [dram.tile([T + 128, 512], bf16, name=f"partial{dn}")
                       for dn in range(2)]
            agin = dram.tile([E * 2 * SHARD], f32)
            agout = dram.tile([2 * T], f32)

            xcT = persist.tile([128, 8, MPAD], bf16)
            hT = persist.tile([128, 32, MPAD], bf16)
            outall = persist.tile([128, 2, NCOLS, 512], bf16)
            idx32 = persist.tile([128, NCOLS], i32)
            idx32s = persist.tile([128, NCOLS], i32)
            w128 = persist.tile([128, NCOLS], f32)
            w2sb1 = persist.tile([128, 32, 512], bf16)

            # ---------- phase 1: local x-slice transpose + router ----------
            with tc.tile_pool(name="p1", bufs=2) as p1, \
                 tc.tile_pool(name="p1ps", bufs=2, space="PSUM") as p1ps, \
                 tc.tile_pool(name="p1ps_s", bufs=1, space="PSUM") as p1ps_s:
                xT = p1.tile([128, 8, SHARD], f32, tag="xT")
                for dk in range(8):
                    nc.sync.dma_start(
                        xT[:, dk, :],
                        xsliceT[:, dk * SHARD:(dk + 1) * SHARD])
                # router: psum_l[tok, jt, e] accumulated over dk
                psum_l = p1ps_s.tile([128, JT, E], f32, tag="psl")
                for jt in range(JT):
                    for dk in range(8):
                        nc.tensor.matmul(
                            psum_l[:, jt, :],
                            xT[:, dk, jt * 128:(jt + 1) * 128],
                            wr_sb[:, dk, :],
                            start=(dk == 0), stop=(dk == 7))
                logits_sb = p1.tile([128, JT, E], f32, tag="logits")
                nc.vector.tensor_copy(logits_sb[:], psum_l[:])
                if has_br:
                    brb_ps = p1ps_s.tile([128, E], f32, tag="brb")
                    nc.tensor.matmul(brb_ps[:], ones_sb[:, :], br_row[:],
                                     start=True, stop=True)
                    nc.vector.tensor_tensor(
                        logits_sb[:], logits_sb[:],
                        brb_ps[:, None, :].to_broadcast([128, JT, E]),
                        mybir.AluOpType.add)

                # ---------- phase 2: top-2 softmax + encode ----------
                maxes = p1.tile([128, JT, 8], f32, tag="maxes")
                for jt in range(JT):
                    nc.vector.max(maxes[:, jt, :], logits_sb[:, jt, :])
                dif = p1.tile([128, JT, E], f32, tag="dif")
                nc.vector.tensor_tensor(
                    dif[:], logits_sb[:],
                    maxes[:, :, 0:1].to_broadcast([128, JT, E]),
                    mybir.AluOpType.subtract)
                ex = p1.tile([128, JT, E], f32, tag="ex")
                nc.scalar.activation(ex[:], dif[:],
                                     mybir.ActivationFunctionType.Exp)
                keep = p1.tile([128, JT, E], f32, tag="keep")
                nc.vector.tensor_tensor(
                    keep[:], logits_sb[:],
                    maxes[:, :, 1:2].to_broadcast([128, JT, E]),
                    mybir.AluOpType.is_ge)
                ek = p1.tile([128, JT, E], f32, tag="ek")
                nc.vector.tensor_tensor(ek[:], ex[:], keep[:],
                                        mybir.AluOpType.mult)
                ssum = p1.tile([128, JT], f32, tag="ssum")
                nc.vector.tensor_reduce(ssum[:], ek[:], mybir.AxisListType.X,
                                        mybir.AluOpType.add)
                rs_t = p1.tile([128, JT], f32, tag="rs_t")
                nc.vector.reciprocal(rs_t[:], ssum[:])
                wgt = p1.tile([128, JT, E], f32, tag="wgt")
                nc.vector.tensor_tensor(
                    wgt[:], ek[:],
                    rs_t[:, :, None].to_broadcast([128, JT, E]),
                    mybir.AluOpType.mult)

                # encode ALL experts: vsel_e = keep_e ? tok : -1,
                # vw_e = keep_e ? w_e : -1; col layout (e, k, j)
                vboth = p1.tile([128, E, 2, JT], f32, tag="vboth")
                nc.vector.tensor_tensor(
                    vboth[:, :, 0, :],
                    keep[:].rearrange("p j e -> p e j"),
                    iota_sb[:, None, :].to_broadcast([128, E, JT]),
                    mybir.AluOpType.mult)
                nc.vector.tensor_tensor(
                    vboth[:, :, 1, :],
                    wgt[:].rearrange("p j e -> p e j"),
                    keep[:].rearrange("p j e -> p e j"),
                    mybir.AluOpType.add)
                vb_flat = vboth[:].rearrange("p e k j -> p (e k j)")
                nc.vector.tensor_scalar(vb_flat, vb_flat, -1.0, None,
                                        op0=mybir.AluOpType.add)

                # fold [128, 64] -> [16, 512] with PE transposes
                ps64 = p1ps_s.tile([64, 128], f32, tag="ps64")
                nc.tensor.transpose(ps64[:], vb_flat, ident[:])
                sb64 = p1.tile([64, 128], f32, tag="sb64")
                nc.vector.tensor_copy(sb64[:], ps64[:])
                vag = p1.tile([16, E, 2, JT, 8], f32, tag="vag")
                for u in range(8):
                    ps16 = p1ps_s.tile([16, 64], f32, tag=f"ps16_{u % 2}",
                                       name=f"ps16_{u}")
                    nc.tensor.transpose(ps16[:],
                                        sb64[:, u * 16:(u + 1) * 16],
                                        ident[:64, :64])
                    nc.vector.tensor_copy(
                        vag[:, :, :, :, u],
                        ps16[:].rearrange("p (e k j) -> p e k j", e=E, k=2))
                nc.sync.dma_start(
                    agin[:].rearrange("(e p c) -> p e c", e=E, p=16),
                    vag[:].rearrange("p e k j u -> p e (k j u)"))

            # zero-fill the RS partial buffers + gather target (issued
            # after the router DMAs so they don't starve the critical path)
            with tc.tile_pool(name="zfill", bufs=1) as zf:
                zrow = zf.tile([128, 512], bf16)
                nc.vector.memset(zrow[:], 0.0)
                for dn in range(2):
                    for j in range(T // 128 + 1):
                        nc.sync.dma_start(
                            partial[dn][j * 128:(j + 1) * 128, :], zrow[:])
                # preload W2's dn=1 half so mm2-dn1 runs DMA-free under RS0
                for hk in range(32):
                    nc.sync.dma_start(w2sb1[:, hk, :], w2a[1, hk])

            # ---------- phase 3: AllToAll + compaction ----------
            nc.gpsimd.collective_compute(
                "AllToAll",
                mybir.AluOpType.bypass,
                replica_groups=[list(range(N_CORES))],
                ins=[agin[:].opt()],
                outs=[agout[:].opt()],
            )

            with tc.tile_pool(name="p3", bufs=1) as p3, \
                 tc.tile_pool(name="p3ps", bufs=1, space="PSUM") as p3ps:
                v16b = p3.tile([16, 2, 8, 32], f32)
                nc.sync.dma_start(
                    v16b[:],
                    agout[:].rearrange("(s p k c) -> p k s c",
                                       s=8, p=16, k=2))
                sg_idx = persist.tile([16, 256], f32)
                sg_w = persist.tile([16, 256], f32)
                nfound = persist.tile([1, 1], u32)
                nfound2 = persist.tile([1, 1], u32)
                sg1 = nc.gpsimd.sparse_gather(
                    sg_idx[:],
                    v16b[:, 0].rearrange("p s c -> p (s c)"),
                    num_found=nfound[:])
                bass._add_dep_helper(sg1.ins, lib_sg.ins, False,
                                     "sparse lib preload")

                # broadcast num_found to 16 partitions via a tiny matmul
                nf_f = p3.tile([1, 1], f32)
                nc.vector.tensor_copy(nf_f[:], nfound[:])
                nf_ps = p3ps.tile([16, 1], f32, tag="nf_ps")
                nc.tensor.matmul(nf_ps[:], ones_sb[:, :16], nf_f[:],
                                 start=True, stop=True)
                nf_b = p3.tile([16, 1], f32)
                nc.vector.tensor_copy(nf_b[:], nf_ps[:])

                valid = persist.tile([16, NIDX], i32)
                nc.vector.tensor_tensor(valid[:], slot_sb[:, :NIDX],
                                        nf_b[:].to_broadcast([16, NIDX]),
                                        mybir.AluOpType.is_lt)
                # int32 gather indices in [128, NCOLS] layout; slots < 896
                # are always valid so fold the raw cast immediately and only
                # route blocks 7-8 through the pad-0 cleanup
                idxri = p3.tile([16, NIDX], i32)
                nc.vector.tensor_copy(idxri[:], sg_idx[:, :NIDX])
                ivr = idxri[:].rearrange("p (c u) -> p c u", u=8)
                for u in range(8):
                    nc.sync.dma_start(idx32[u * 16:(u + 1) * 16, :],
                                      ivr[:, :, u])
                idx0 = p3.tile([16, NIDX], f32)
                nc.vector.memset(idx0[:], 0.0)
                nc.vector.copy_predicated(idx0[:], valid[:],
                                          sg_idx[:, :NIDX])
                idx0i = p3.tile([16, NIDX], i32)
                nc.vector.tensor_copy(idx0i[:], idx0[:])
                iv = idx0i[:].rearrange("p (c u) -> p c u", u=8)
                for u in range(8):
                    nc.sync.dma_start(idx32[u * 16:(u + 1) * 16, 7:NCOLS],
                                      iv[:, 7:NCOLS, u])
                if os.environ.get('KDEBUG') == '1':
                    idxf_dbg = p3.tile([16, NIDX], f32)
                    nc.vector.memset(idxf_dbg[:], -1.0)
                    nc.vector.copy_predicated(idxf_dbg[:], valid[:],
                                              sg_idx[:, :NIDX])
                    nc.sync.dma_start(out_shard[0:16, 0:NIDX], idxf_dbg[:])
                    nc.sync.dma_start(out_shard[16:32, 0:NIDX], wcl[:])
                    nf_dbg = p3.tile([1, 1], f32)
                    nc.vector.tensor_copy(nf_dbg[:], nfound[:])
                    nc.sync.dma_start(out_shard[32:33, 0:1], nf_dbg[:])

            # ---------- phase 4: gather selected tokens + transpose ----------
            with tc.tile_pool(name="p4", bufs=1) as p4, \
                 tc.tile_pool(name="p4ps", bufs=4, space="PSUM") as p4ps:
                xg = p4.tile([128, NCOLS, D], f32)
                for b in range(NCOLS):
                    nc.gpsimd.indirect_dma_start(
                        out=xg[:, b, :], out_offset=None,
                        in_=x[:],
                        in_offset=bass.IndirectOffsetOnAxis(
                            ap=idx32[:, b:b + 1], axis=0))
                # weight stream compaction + tables (needed only from mm2 on)
                sg2 = nc.gpsimd.sparse_gather(
                    sg_w[:],
                    v16b[:, 1].rearrange("p s c -> p (s c)"),
                    num_found=nfound2[:])
                wcl = p4.tile([16, NIDX], f32)
                nc.vector.memset(wcl[:], 0.0)
                nc.vector.copy_predicated(wcl[:], valid[:], sg_w[:, :NIDX])
                wv = wcl[:].rearrange("p (c u) -> p c u", u=8)
                for u in range(8):
                    nc.sync.dma_start(w128[u * 16:(u + 1) * 16, :],
                                      wv[:, :, u])
                idxs_f = p4.tile([16, NIDX], f32)
                nc.vector.memset(idxs_f[:], float(T))
                nc.vector.copy_predicated(idxs_f[:], valid[:],
                                          sg_idx[:, :NIDX])
                idxs_i = p4.tile([16, NIDX], i32)
                nc.vector.tensor_copy(idxs_i[:], idxs_f[:])
                sv = idxs_i[:].rearrange("p (c u) -> p c u", u=8)
                for u in range(8):
                    nc.sync.dma_start(idx32s[u * 16:(u + 1) * 16, :],
                                      sv[:, :, u])
                for b in range(NCOLS):
                    for dk4 in range(2):
                        pst2 = p4ps.tile([128, 512], f32, tag="pst2")
                        for q in range(4):
                            dk = dk4 * 4 + q
                            nc.tensor.transpose(
                                pst2[:, q * 128:(q + 1) * 128],
                                xg[:, b, dk * 128:(dk + 1) * 128], ident[:])
                        for q in range(4):
                            dk = dk4 * 4 + q
                            nc.vector.tensor_copy(
                                xcT[:, dk, b * 128:(b + 1) * 128],
                                pst2[:, q * 128:(q + 1) * 128])

            # ---------- phase 5: mm1 (hT = gelu(W1^T xc^T + b1)) ----------
            CH = [(0, 512), (512, 512), (1024, 64)]
            with tc.tile_pool(name="p5", bufs=4) as p5, \
                 tc.tile_pool(name="p5ps", bufs=2, space="PSUM") as p5ps:
                for hm in range(32):
                    w1bf = p5.tile([128, 8, 128], bf16, tag="w1bf")
                    nc.sync.dma_start(
                        w1bf[:].rearrange("p a b -> p (a b)"), w1a[hm])
                    psums = [p5ps.tile([128, cn], f32, tag=f"mm1_{s}",
                                       name=f"mm1ps_{hm}_{s}")
                             for s, (c0, cn) in enumerate(CH)]
                    for dk in range(8):
                        for s, (c0, cn) in enumerate(CH):
                            nc.tensor.matmul(
                                psums[s][:], w1bf[:, dk, :],
                                xcT[:, dk, c0:c0 + cn],
                                start=(dk == 0), stop=(dk == 7))
                    for s, (c0, cn) in enumerate(CH):
                        nc.scalar.activation(
                            hT[:, hm, c0:c0 + cn], psums[s][:],
                            mybir.ActivationFunctionType.Gelu,
                            bias=b1_sb[:, hm:hm + 1])

            # ---------- phase 6: mm2 + weight + scatter + RS (per D-half) ----
            rs_out = [dram.tile([SHARD, 512], bf16, name=f"rs{dn}")
                      for dn in range(2)]
            TBG = [(0, 5), (5, 4)]
            with tc.tile_pool(name="p6", bufs=8) as p6, \
                 tc.tile_pool(name="p6o", bufs=2) as p6o, \
                 tc.tile_pool(name="p6ps", bufs=1, space="PSUM") as p6ps:
                for dn in range(2):
                    for tb0, tbn in TBG:
                        psum_o = {}
                        for tb in range(tb0, tb0 + tbn):
                            psum_o[tb] = p6ps.tile(
                                [128, 512], f32, tag=f"mm2_{tb - tb0}",
                                name=f"mm2ps_{dn}_{tb}")
                        for hk in range(32):
                            if dn == 0:
                                w2bf = p6.tile([128, 512], bf16, tag="w2bf")
                                nc.sync.dma_start(w2bf[:], w2a[0, hk])
                                w2op = w2bf[:]
                            else:
                                w2op = w2sb1[:, hk, :]
                            for tb in range(tb0, tb0 + tbn):
                                nc.tensor.matmul(
                                    psum_o[tb],
                                    hT[:, hk, tb * 128:(tb + 1) * 128],
                                    w2op,
                                    start=(hk == 0), stop=(hk == 31))
                        for tb in range(tb0, tb0 + tbn):
                            if has_b2:
                                outf = p6o.tile([128, 512], f32, tag="outf")
                                nc.vector.tensor_scalar_mul(
                                    outf[:], psum_o[tb], w128[:, tb:tb + 1])
                                b2sb = p6o.tile([1, 512], f32, tag="b2sb")
                                nc.sync.dma_start(
                                    b2sb[:], b2s[None,
                                                 dn * 512:(dn + 1) * 512])
                                b2ps = p6ps.tile([128, 512], f32, tag="b2ps")
                                nc.tensor.matmul(
                                    b2ps[:], ones_sb[:, :], b2sb[:],
                                    start=True, stop=True)
                                b2w = p6o.tile([128, 512], f32, tag="b2w")
                                nc.vector.tensor_scalar_mul(
                                    b2w[:], b2ps[:], w128[:, tb:tb + 1])
                                nc.vector.tensor_tensor(
                                    outf[:], outf[:], b2w[:],
                                    mybir.AluOpType.add)
                                nc.vector.tensor_copy(
                                    outall[:, dn, tb, :], outf[:])
                            else:
                                nc.vector.tensor_scalar_mul(
                                    outall[:, dn, tb, :], psum_o[tb],
                                    w128[:, tb:tb + 1])
                            # scatter-write this block (rows unique per core;
                            # pad slots land on trash row T)
                            nc.gpsimd.indirect_dma_start(
                                out=partial[dn][:],
                                out_offset=bass.IndirectOffsetOnAxis(
                                    ap=idx32s[:, tb:tb + 1], axis=0),
                                in_=outall[:, dn, tb, :], in_offset=None)
                    nc.gpsimd.collective_compute(
                        "ReduceScatter",
                        mybir.AluOpType.add,
                        replica_groups=[list(range(N_CORES))],
                        ins=[partial[dn][:T].opt()],
                        outs=[rs_out[dn][:].opt()],
                    )

            # ---------- phase 7: emit fp32 output shard ----------
            # (dn=0 pass depends only on rs_out[0], so it overlaps RS1)
            with tc.tile_pool(name="p7", bufs=4) as p7:
                for dn in range(2):
                    for j in range(SHARD // 128):
                        ob = p7.tile([128, 512], bf16, tag="ob")
                        nc.sync.dma_start(
                            ob[:], rs_out[dn][j * 128:(j + 1) * 128, :])
                        of = p7.tile([128, 512], f32, tag="of")
                        nc.scalar.activation(
                            of[:], ob[:],
                            mybir.ActivationFunctionType.Identity)
                        nc.sync.dma_start(
                            out_shard[j * 128:(j + 1) * 128,
                                      dn * 512:(dn + 1) * 512], of[:])

    nc.compile()
    return nc


def _get_kernel(has_br: bool, has_b2: bool, reps: int = 1):
    key = (has_br, has_b2, reps)
    if key not in _kernel_cache:
        _kernel_cache[key] = _build(has_br, has_b2, reps)
    return _kernel_cache[key]


def _const_inputs():
    identc = np.eye(128, dtype=np.float32)
    slotio = (np.arange(256)[None, :] * 16
              + np.arange(16)[:, None]).astype(np.float32)
    onesrow = np.ones((1, 128), np.float32)
    return identc, slotio, onesrow


def make_in_maps(x, W1, b1, W2, b2, Wr, br):
    xf = np.ascontiguousarray(np.asarray(x, np.float32).reshape(T, D))
    W1 = np.asarray(W1, dtype=np.float32).astype(ml_dtypes.bfloat16)
    b1 = np.asarray(b1, dtype=np.float32)
    W2 = np.asarray(W2, dtype=np.float32).astype(ml_dtypes.bfloat16)
    b2 = np.asarray(b2, dtype=np.float32)
    Wr = np.ascontiguousarray(np.asarray(Wr, dtype=np.float32))
    br = np.ascontiguousarray(np.asarray(br, dtype=np.float32))
    identc, slotio, onesrow = _const_inputs()
    in_maps = []
    for r in range(N_CORES):
        iota = (512 * r + np.arange(JT)[None, :] * 128
                + np.arange(128)[:, None] + 1.0).astype(np.float32)
        xs = xf[512 * r:512 * (r + 1)]                       # [512, 1024]
        xsT = np.ascontiguousarray(
            xs.T.reshape(8, 128, SHARD).transpose(1, 0, 2)
            .reshape(128, 8 * SHARD))
        w1r = np.ascontiguousarray(
            W1[r].reshape(8, 128, 32, 128).transpose(2, 1, 0, 3)
            .reshape(32, 128, 8 * 128))
        w2r = np.ascontiguousarray(
            W2[r].reshape(32, 128, 2, 512).transpose(2, 0, 1, 3))
        in_maps.append({
            "x": xf,
            "xsliceT": xsT,
            "w1a": w1r,
            "b1s": np.ascontiguousarray(b1[r]),
            "w2a": w2r,
            "b2s": np.ascontiguousarray(b2[r]),
            "wr": Wr,
            "br": br,
            "identc": identc,
            "iota4": iota,
            "slotio": slotio,
            "onesrow": onesrow,
        })
    return in_maps


def kernel(x, W1, b1, W2, b2, Wr, br):
    x = np.asarray(x, dtype=np.float32)
    B, S, _ = x.shape
    has_br = bool(np.any(np.asarray(br)))
    has_b2 = bool(np.any(np.asarray(b2)))
    nc = _get_kernel(has_br, has_b2)
    in_maps = make_in_maps(x, W1, b1, W2, b2, Wr, br)
    res = bass_utils.run_bass_kernel_spmd(
        nc, in_maps, core_ids=list(range(N_CORES)))
    out = np.concatenate([res.results[r]["out_shard"] for r in range(N_CORES)],
                         axis=0)
    return out.reshape(B, S, D)



# revision 5
# speedup vs baseline: 1.5084x; 1.5084x over previous
"""MoE block (D=1024, H=4096, E=8, top-2) on 8 Trainium2 NeuronCores.

Strategy: expert-parallel with a sharded router.
Core r owns expert r (W1[r]/b1[r]/W2[r]/b2[r] shipped pre-cast to bf16) and
routes only its own 512-token shard of x:
  1. loads its x-slice pre-transposed, computes router logits [tok, E] in
     fp32, does the top-2 threshold softmax, and encodes per-expert
     (index, weight) streams in a 16-partition-wrapped layout (built with
     two PE transposes, no elementwise DMA),
  2. AllToAll ships each expert's stream to its owner core (32 KB), then
     GPSIMD sparse_gather compacts the <=1152 selected tokens; the count is
     loaded into Pool registers and the cleaned (pad = -1) index stream is
     replicated to all 8 Q7 cores via one PE matmul,
  3. GPSIMD dma_gather(transpose=True) fetches the selected token rows from
     a bf16 copy of x directly into [D-part, slot] layout (3 chunks of 384
     slots each, pipelined into mm1; no PE transposes needed),
  4. expert FFN in bf16 (fp32 accumulate): hT = gelu(W1^T xc^T + b1) per
     384-slot chunk; mm2 is split into two D-halves x three 384-slot groups,
     each group is routing-weight scaled and dma_scatter_add'ed into a
     zero-filled [T, 512] bf16 partial (pad slots skipped via the count
     registers), then ReduceScattered over the 8 cores -- the first RS
     overlaps the second half's matmuls,
  5. the two RS outputs land directly in bf16 ExternalOutputs; the host
     concatenates and upcasts core r's token rows [512*r : 512*(r+1)].
"""

import sys
import numpy as np
import ml_dtypes

sys.path.insert(0, "/opt/trn_rl_repo")

import concourse.bass as bass            # noqa: E402
import concourse.mybir as mybir          # noqa: E402
import concourse.tile as tile            # noqa: E402
from concourse import bacc               # noqa: E402
from concourse import bass_utils         # noqa: E402
from concourse import library_config      # noqa: E402

T, D, H, E = 4096, 1024, 4096, 8
N_CORES = 8
MPAD = 1152
CK = 384                     # slot chunk for dma_gather / scatter_add
NCH = MPAD // CK             # 3
NIDX = MPAD // 16            # 72
SHARD = T // N_CORES         # 512
JT = SHARD // 128            # 4

f32 = mybir.dt.float32
bf16 = mybir.dt.bfloat16
i32 = mybir.dt.int32
i16 = mybir.dt.int16
u32 = mybir.dt.uint32

_kernel_cache = {}


def _build(has_br: bool, has_b2: bool, reps: int = 1):
    nc = bacc.Bacc("TRN2", target_bir_lowering=False, debug=False,
                   num_devices=N_CORES)
    xbf = nc.dram_tensor("xbf", [T, D], bf16, kind="ExternalInput")
    xsliceT = nc.dram_tensor("xsliceT", [128, 8 * SHARD], f32,
                             kind="ExternalInput")
    w1a = nc.dram_tensor("w1a", [32, 128, 8 * 128], bf16,
                         kind="ExternalInput")
    b1s = nc.dram_tensor("b1s", [H], f32, kind="ExternalInput")
    w2a = nc.dram_tensor("w2a", [2, 32, 128, 512], bf16,
                         kind="ExternalInput")
    b2s = nc.dram_tensor("b2s", [D], f32, kind="ExternalInput")
    wr = nc.dram_tensor("wr", [D, E], f32, kind="ExternalInput")
    br = nc.dram_tensor("br", [E], f32, kind="ExternalInput")
    identc = nc.dram_tensor("identc", [128, 128], f32, kind="ExternalInput")
    iota4 = nc.dram_tensor("iota4", [128, JT], f32, kind="ExternalInput")
    slotio = nc.dram_tensor("slotio", [16, 256], f32, kind="ExternalInput")
    onesrow = nc.dram_tensor("onesrow", [1, 128], f32, kind="ExternalInput")
    repm = nc.dram_tensor("repm", [16, 128], f32, kind="ExternalInput")
    out0 = nc.dram_tensor("out0", [SHARD, 512], bf16, kind="ExternalOutput")
    out1 = nc.dram_tensor("out1", [SHARD, 512], bf16, kind="ExternalOutput")
    outs = [out0, out1]

    with tile.TileContext(nc) as tc:
        with tc.tile_pool(name="persist", bufs=1) as persist, \
             tc.tile_pool(name="dram", bufs=1, space="DRAM") as dram:

            ident = persist.tile([128, 128], f32)
            nc.sync.dma_start(ident[:], identc[:])
            wr_sb = persist.tile([128, 8, E], f32)
            nc.sync.dma_start(wr_sb[:], wr[:].rearrange("(o p) e -> p o e", p=128))
            b1_sb = persist.tile([128, 32], f32)
            nc.sync.dma_start(b1_sb[:], b1s[:].rearrange("(o p) -> p o", p=128))
            ones_sb = persist.tile([1, 128], f32)
            nc.sync.dma_start(ones_sb[:], onesrow[:])
            iota_sb = persist.tile([128, JT], f32)
            nc.sync.dma_start(iota_sb[:], iota4[:])
            slot_sb = persist.tile([16, 256], f32)
            nc.sync.dma_start(slot_sb[:], slotio[:])
            repm_sb = persist.tile([16, 128], f32)
            nc.sync.dma_start(repm_sb[:], repm[:])
            if has_br:
                br_row = persist.tile([1, E], f32)
                nc.sync.dma_start(br_row[:], br[None, :])

            lib_sg = nc.gpsimd.load_library(library_config.sparse_gather)

            partial = [dram.tile([T, 512], bf16, name=f"partial{dn}")
                       for dn in range(2)]
            rs_out = [dram.tile([SHARD, 512], bf16, name=f"rs{dn}")
                      for dn in range(2)]
            agin = dram.tile([E * 2 * SHARD], f32)
            agout = dram.tile([2 * T], f32)

            xcT = persist.tile([128, NCH, 8, CK], bf16)
            hT = persist.tile([128, 32, MPAD], bf16)
            outall = persist.tile([128, 2, MPAD // 128, 512], bf16)
            idx16 = persist.tile([128, NIDX], i16)
            w128 = persist.tile([128, MPAD // 128], f32)
            w2sb1 = persist.tile([128, 32, 512], bf16)

            # ---------- phase 1: local x-slice transpose + router ----------
            with tc.tile_pool(name="p1", bufs=2) as p1, \
                 tc.tile_pool(name="p1ps_s", bufs=1, space="PSUM") as p1ps_s:
                xT = p1.tile([128, 8, SHARD], f32, tag="xT")
                for dk in range(8):
                    nc.sync.dma_start(
                        xT[:, dk, :],
                        xsliceT[:, dk * SHARD:(dk + 1) * SHARD])
                # router: psum_l[tok, jt, e] accumulated over dk
                psum_l = p1ps_s.tile([128, JT, E], f32, tag="psl")
                for jt in range(JT):
                    for dk in range(8):
                        nc.tensor.matmul(
                            psum_l[:, jt, :],
                            xT[:, dk, jt * 128:(jt + 1) * 128],
                            wr_sb[:, dk, :],
                            start=(dk == 0), stop=(dk == 7))
                logits_sb = p1.tile([128, JT, E], f32, tag="logits")
                nc.vector.tensor_copy(logits_sb[:], psum_l[:])
                if has_br:
                    brb_ps = p1ps_s.tile([128, E], f32, tag="brb")
                    nc.tensor.matmul(brb_ps[:], ones_sb[:, :], br_row[:],
                                     start=True, stop=True)
                    nc.vector.tensor_tensor(
                        logits_sb[:], logits_sb[:],
                        brb_ps[:, None, :].to_broadcast([128, JT, E]),
                        mybir.AluOpType.add)

                # ---------- phase 2: top-2 softmax + encode ----------
                maxes = p1.tile([128, JT, 8], f32, tag="maxes")
                for jt in range(JT):
                    nc.vector.max(maxes[:, jt, :], logits_sb[:, jt, :])
                dif = p1.tile([128, JT, E], f32, tag="dif")
                nc.vector.tensor_tensor(
                    dif[:], logits_sb[:],
                    maxes[:, :, 0:1].to_broadcast([128, JT, E]),
                    mybir.AluOpType.subtract)
                ex = p1.tile([128, JT, E], f32, tag="ex")
                nc.scalar.activation(ex[:], dif[:],
                                     mybir.ActivationFunctionType.Exp)
                keep = p1.tile([128, JT, E], f32, tag="keep")
                nc.vector.tensor_tensor(
                    keep[:], logits_sb[:],
                    maxes[:, :, 1:2].to_broadcast([128, JT, E]),
                    mybir.AluOpType.is_ge)
                ek = p1.tile([128, JT, E], f32, tag="ek")
                nc.vector.tensor_tensor(ek[:], ex[:], keep[:],
                                        mybir.AluOpType.mult)
                ssum = p1.tile([128, JT], f32, tag="ssum")
                nc.vector.tensor_reduce(ssum[:], ek[:], mybir.AxisListType.X,
                                        mybir.AluOpType.add)
                rs_t = p1.tile([128, JT], f32, tag="rs_t")
                nc.vector.reciprocal(rs_t[:], ssum[:])
                wgt = p1.tile([128, JT, E], f32, tag="wgt")
                nc.vector.tensor_tensor(
                    wgt[:], ek[:],
                    rs_t[:, :, None].to_broadcast([128, JT, E]),
                    mybir.AluOpType.mult)

                # encode ALL experts: vsel_e = keep_e ? tok : -1,
                # vw_e = keep_e ? w_e : -1; col layout (e, k, j)
                vboth = p1.tile([128, E, 2, JT], f32, tag="vboth")
                nc.vector.tensor_tensor(
                    vboth[:, :, 0, :],
                    keep[:].rearrange("p j e -> p e j"),
                    iota_sb[:, None, :].to_broadcast([128, E, JT]),
                    mybir.AluOpType.mult)
                nc.vector.tensor_tensor(
                    vboth[:, :, 1, :],
                    wgt[:].rearrange("p j e -> p e j"),
                    keep[:].rearrange("p j e -> p e j"),
                    mybir.AluOpType.add)
                vb_flat = vboth[:].rearrange("p e k j -> p (e k j)")
                nc.vector.tensor_scalar(vb_flat, vb_flat, -1.0, None,
                                        op0=mybir.AluOpType.add)

                # fold [128, 64] -> [16, 512] with PE transposes
                ps64 = p1ps_s.tile([64, 128], f32, tag="ps64")
                nc.tensor.transpose(ps64[:], vb_flat, ident[:])
                sb64 = p1.tile([64, 128], f32, tag="sb64")
                nc.vector.tensor_copy(sb64[:], ps64[:])
                vag = p1.tile([16, E, 2, JT, 8], f32, tag="vag")
                for u in range(8):
                    ps16 = p1ps_s.tile([16, 64], f32, tag=f"ps16_{u % 2}",
                                       name=f"ps16_{u}")
                    nc.tensor.transpose(ps16[:],
                                        sb64[:, u * 16:(u + 1) * 16],
                                        ident[:64, :64])
                    nc.vector.tensor_copy(
                        vag[:, :, :, :, u],
                        ps16[:].rearrange("p (e k j) -> p e k j", e=E, k=2))
                nc.sync.dma_start(
                    agin[:].rearrange("(e p c) -> p e c", e=E, p=16),
                    vag[:].rearrange("p e k j u -> p e (k j u)"))

            # ---------- phase 3: AllToAll + compaction ----------
            nc.gpsimd.collective_compute(
                "AllToAll",
                mybir.AluOpType.bypass,
                replica_groups=[list(range(N_CORES))],
                ins=[agin[:].opt()],
                outs=[agout[:].opt()],
            )

            nfr = nc.alloc_register(mybir.EngineType.Pool, "nfr")
            cregs = [nc.alloc_register(mybir.EngineType.Pool, f"cr{k}")
                     for k in range(NCH)]

            with tc.tile_pool(name="p3", bufs=1) as p3, \
                 tc.tile_pool(name="p3ps", bufs=1, space="PSUM") as p3ps:
                v16b = p3.tile([16, 2, 8, 32], f32)
                nc.sync.dma_start(
                    v16b[:],
                    agout[:].rearrange("(s p k c) -> p k s c",
                                       s=8, p=16, k=2))
                sg_idx = p3.tile([16, 256], f32)
                sg_w = persist.tile([16, 256], f32)
                nfound = p3.tile([1, 1], u32)
                nfound2 = p3.tile([1, 1], u32)
                sg1 = nc.gpsimd.sparse_gather(
                    sg_idx[:],
                    v16b[:, 0].rearrange("p s c -> p (s c)"),
                    num_found=nfound[:])
                bass._add_dep_helper(sg1.ins, lib_sg.ins, False,
                                     "sparse lib preload")
                nc.gpsimd.sparse_gather(
                    sg_w[:],
                    v16b[:, 1].rearrange("p s c -> p (s c)"),
                    num_found=nfound2[:])
                # per-chunk valid counts into Pool registers
                nc.gpsimd.reg_load(nfr, nfound[:1, :1])
                for k in range(NCH):
                    if k == 0:
                        nc.gpsimd.reg_alu(cregs[0], nfr, CK,
                                          mybir.AluOpType.min)
                    else:
                        nc.gpsimd.reg_alu(cregs[k], nfr, CK * k,
                                          mybir.AluOpType.subtract)
                        nc.gpsimd.reg_alu(cregs[k], cregs[k], CK,
                                          mybir.AluOpType.min)
                lib_mlp = nc.gpsimd.load_library(library_config.mlp)

                # broadcast num_found to 16 partitions via a tiny matmul
                nf_f = p3.tile([1, 1], f32)
                nc.vector.tensor_copy(nf_f[:], nfound[:])
                nf_ps = p3ps.tile([16, 1], f32, tag="nf_ps")
                nc.tensor.matmul(nf_ps[:], ones_sb[:, :16], nf_f[:],
                                 start=True, stop=True)
                nf_b = p3.tile([16, 1], f32)
                nc.vector.tensor_copy(nf_b[:], nf_ps[:])
                valid = p3.tile([16, NIDX], i32)
                nc.vector.tensor_tensor(valid[:], slot_sb[:, :NIDX],
                                        nf_b[:].to_broadcast([16, NIDX]),
                                        mybir.AluOpType.is_lt)
                # clean pads to -1, then replicate to the 8 Q7 core groups
                # via one PE matmul and cast to int16
                idxm = p3.tile([16, NIDX], f32)
                nc.vector.memset(idxm[:], -1.0)
                nc.vector.copy_predicated(idxm[:], valid[:],
                                          sg_idx[:, :NIDX])
                idx_ps = p3ps.tile([128, NIDX], f32, tag="idx_ps")
                nc.tensor.matmul(idx_ps[:], repm_sb[:], idxm[:],
                                 start=True, stop=True)
                nc.vector.tensor_copy(idx16[:], idx_ps[:])

                # weight table spread [16, 72] -> [128, 9] (pad slots are
                # never scattered, so no cleanup needed)
                wv = sg_w[:].rearrange("p (c u) -> p c u", u=8)
                for u in range(8):
                    nc.sync.dma_start(w128[u * 16:(u + 1) * 16, :],
                                      wv[:, :MPAD // 128, u])

                # ---------- phase 4: gather selected tokens (transposed) ----
                for k in range(NCH):
                    g = nc.gpsimd.dma_gather(
                        xcT[:, k], xbf[:],
                        idx16[:, 24 * k:24 * (k + 1)], CK, cregs[k], D,
                        transpose=True)
                    if k == 0:
                        bass._add_dep_helper(g.ins, lib_mlp.ins, False,
                                             "mlp lib preload")

            # zero-fill the RS partial buffers + preload W2 dn=1
            # (program order places these after the critical gather issue)
            with tc.tile_pool(name="zfill", bufs=1) as zf:
                zrow = zf.tile([128, 512], bf16)
                nc.vector.memset(zrow[:], 0.0)
                for dn in range(2):
                    for j in range(T // 128):
                        nc.sync.dma_start(
                            partial[dn][j * 128:(j + 1) * 128, :], zrow[:])
                for hk in range(32):
                    nc.sync.dma_start(w2sb1[:, hk, :], w2a[1, hk])

            # ---------- phase 5: mm1 (hT = gelu(W1^T xc^T + b1)) ----------
            with tc.tile_pool(name="p5", bufs=4) as p5, \
                 tc.tile_pool(name="p5ps", bufs=3, space="PSUM") as p5ps:
                for hm in range(32):
                    w1bf = p5.tile([128, 8, 128], bf16, tag="w1bf",
                                   name=f"w1bf_{hm}")
                    nc.scalar.dma_start(
                        w1bf[:].rearrange("p a b -> p (a b)"), w1a[hm])
                    for k in range(NCH):
                        psum = p5ps.tile([128, CK], f32, tag="mm1",
                                         name=f"mm1ps_{k}_{hm}")
                        for dk in range(8):
                            nc.tensor.matmul(
                                psum[:], w1bf[:, dk, :],
                                xcT[:, k, dk, :],
                                start=(dk == 0), stop=(dk == 7))
                        nc.scalar.activation(
                            hT[:, hm, k * CK:(k + 1) * CK], psum[:],
                            mybir.ActivationFunctionType.Gelu,
                            bias=b1_sb[:, hm:hm + 1])

            # ---------- phase 6: mm2 + weight + scatter-add + RS ----------
            with tc.tile_pool(name="p6", bufs=8) as p6, \
                 tc.tile_pool(name="p6o", bufs=2) as p6o, \
                 tc.tile_pool(name="p6ps", bufs=1, space="PSUM") as p6ps:
                for dn in range(2):
                    for g in range(NCH):
                        psum_o = {}
                        for tb in range(3 * g, 3 * g + 3):
                            psum_o[tb] = p6ps.tile(
                                [128, 512], f32, tag=f"mm2_{tb - 3 * g}",
                                name=f"mm2ps_{dn}_{tb}")
                        for hk in range(32):
                            if dn == 0:
                                w2bf = p6.tile([128, 512], bf16, tag="w2bf",
                                               name=f"w2bf_{g}_{hk}")
                                nc.scalar.dma_start(w2bf[:], w2a[0, hk])
                                w2op = w2bf[:]
                            else:
                                w2op = w2sb1[:, hk, :]
                            for tb in range(3 * g, 3 * g + 3):
                                nc.tensor.matmul(
                                    psum_o[tb],
                                    hT[:, hk, tb * 128:(tb + 1) * 128],
                                    w2op,
                                    start=(hk == 0), stop=(hk == 31))
                        for tb in range(3 * g, 3 * g + 3):
                            if has_b2:
                                outf = p6o.tile([128, 512], f32, tag="outf")
                                nc.vector.tensor_scalar_mul(
                                    outf[:], psum_o[tb], w128[:, tb:tb + 1])
                                b2sb = p6o.tile([1, 512], f32, tag="b2sb")
                                nc.sync.dma_start(
                                    b2sb[:], b2s[None,
                                                 dn * 512:(dn + 1) * 512])
                                b2ps = p6ps.tile([128, 512], f32, tag="b2ps")
                                nc.tensor.matmul(
                                    b2ps[:], ones_sb[:, :], b2sb[:],
                                    start=True, stop=True)
                                b2w = p6o.tile([128, 512], f32, tag="b2w")
                                nc.vector.tensor_scalar_mul(
                                    b2w[:], b2ps[:], w128[:, tb:tb + 1])
                                nc.vector.tensor_tensor(
                                    outf[:], outf[:], b2w[:],
                                    mybir.AluOpType.add)
                                nc.vector.tensor_copy(
                                    outall[:, dn, tb, :], outf[:])
                            else:
                                nc.vector.tensor_scalar_mul(
                                    outall[:, dn, tb, :], psum_o[tb],
                                    w128[:, tb:tb + 1])
                        nc.gpsimd.dma_scatter_add(
                            partial[dn][:],
                            outall[:, dn, 3 * g:3 * g + 3, :],
                            idx16[:, 24 * g:24 * (g + 1)],
                            CK, cregs[g], 512)
                    nc.gpsimd.collective_compute(
                        "ReduceScatter",
                        mybir.AluOpType.add,
                        replica_groups=[list(range(N_CORES))],
                        ins=[partial[dn][:].opt()],
                        outs=[rs_out[dn][:].opt()],
                    )
                    nc.sync.dma_start(outs[dn][:], rs_out[dn][:])

    nc.compile()
    return nc


def _get_kernel(has_br: bool, has_b2: bool, reps: int = 1):
    key = (has_br, has_b2, reps)
    if key not in _kernel_cache:
        _kernel_cache[key] = _build(has_br, has_b2, reps)
    return _kernel_cache[key]


def _const_inputs():
    identc = np.eye(128, dtype=np.float32)
    slotio = (np.arange(256)[None, :] * 16
              + np.arange(16)[:, None]).astype(np.float32)
    onesrow = np.ones((1, 128), np.float32)
    repm = np.tile(np.eye(16, dtype=np.float32), (1, 8))
    return identc, slotio, onesrow, repm


def make_in_maps(x, W1, b1, W2, b2, Wr, br):
    xf = np.ascontiguousarray(np.asarray(x, np.float32).reshape(T, D))
    xbf = np.ascontiguousarray(xf.astype(ml_dtypes.bfloat16))
    W1 = np.asarray(W1, dtype=np.float32).astype(ml_dtypes.bfloat16)
    b1 = np.asarray(b1, dtype=np.float32)
    W2 = np.asarray(W2, dtype=np.float32).astype(ml_dtypes.bfloat16)
    b2 = np.asarray(b2, dtype=np.float32)
    Wr = np.ascontiguousarray(np.asarray(Wr, dtype=np.float32))
    br = np.ascontiguousarray(np.asarray(br, dtype=np.float32))
    identc, slotio, onesrow, repm = _const_inputs()
    in_maps = []
    for r in range(N_CORES):
        iota = (512 * r + np.arange(JT)[None, :] * 128
                + np.arange(128)[:, None] + 1.0).astype(np.float32)
        xs = xf[512 * r:512 * (r + 1)]                       # [512, 1024]
        xsT = np.ascontiguousarray(
            xs.T.reshape(8, 128, SHARD).transpose(1, 0, 2)
            .reshape(128, 8 * SHARD))
        w1r = np.ascontiguousarray(
            W1[r].reshape(8, 128, 32, 128).transpose(2, 1, 0, 3)
            .reshape(32, 128, 8 * 128))
        w2r = np.ascontiguousarray(
            W2[r].reshape(32, 128, 2, 512).transpose(2, 0, 1, 3))
        in_maps.append({
            "xbf": xbf,
            "xsliceT": xsT,
            "w1a": w1r,
            "b1s": np.ascontiguousarray(b1[r]),
            "w2a": w2r,
            "b2s": np.ascontiguousarray(b2[r]),
            "wr": Wr,
            "br": br,
            "identc": identc,
            "iota4": iota,
            "slotio": slotio,
            "onesrow": onesrow,
            "repm": repm,
        })
    return in_maps


def kernel(x, W1, b1, W2, b2, Wr, br):
    x = np.asarray(x, dtype=np.float32)
    B, S, _ = x.shape
    has_br = bool(np.any(np.asarray(br)))
    has_b2 = bool(np.any(np.asarray(b2)))
    nc = _get_kernel(has_br, has_b2)
    in_maps = make_in_maps(x, W1, b1, W2, b2, Wr, br)
    res = bass_utils.run_bass_kernel_spmd(
        nc, in_maps, core_ids=list(range(N_CORES)))
    out = np.concatenate(
        [np.concatenate([np.asarray(res.results[r]["out0"]),
                         np.asarray(res.results[r]["out1"])], axis=1)
         for r in range(N_CORES)], axis=0)
    return out.astype(np.float32).reshape(B, S, D)


# revision 13
# speedup vs baseline: 1.7004x; 1.1273x over previous
"""MoE block (D=1024, H=4096, E=8, top-2) on 8 Trainium2 NeuronCores.

Strategy: expert-parallel with a sharded router.
Core r owns expert r (W1[r]/b1[r]/W2[r]/b2[r] shipped pre-cast to bf16) and
routes only its own 512-token shard of x:
  1. loads its x-slice pre-transposed, computes router logits [tok, E] in
     fp32, does the top-2 threshold softmax, and encodes per-expert
     (index, weight) streams in a 16-partition-wrapped layout (built with
     two PE transposes, no elementwise DMA),
  2. AllToAll ships each expert's stream to its owner core (32 KB), then
     GPSIMD sparse_gather compacts the <=1152 selected tokens; the count is
     loaded into Pool registers and the cleaned (pad = -1) index stream is
     replicated to all 8 Q7 cores via one PE matmul,
  3. GPSIMD dma_gather(transpose=True) fetches the selected token rows from
     a bf16 copy of x directly into [D-part, slot] layout (3 chunks of 384
     slots each, pipelined into mm1; no PE transposes needed),
  4. expert FFN in bf16 (fp32 accumulate): hT = gelu(W1^T xc^T + b1) per
     384-slot chunk; mm2 is split into two D-halves x three 384-slot groups,
     each group is routing-weight scaled and dma_scatter_add'ed into a
     zero-filled [T, 512] bf16 partial (pad slots skipped via the count
     registers), then ReduceScattered over the 8 cores -- the first RS
     overlaps the second half's matmuls,
  5. the two RS outputs land directly in bf16 ExternalOutputs; the host
     concatenates and upcasts core r's token rows [512*r : 512*(r+1)].
"""

import sys
import numpy as np
import ml_dtypes

sys.path.insert(0, "/opt/trn_rl_repo")

import concourse.bass as bass            # noqa: E402
import concourse.mybir as mybir          # noqa: E402
import concourse.tile as tile            # noqa: E402
from concourse import bacc               # noqa: E402
from concourse import bass_utils         # noqa: E402
from concourse import library_config      # noqa: E402

T, D, H, E = 4096, 1024, 4096, 8
N_CORES = 8
MPAD = 1152
CK = 384                     # slot chunk for dma_gather / scatter_add
NCH = MPAD // CK             # 3
NIDX = MPAD // 16            # 72
SHARD = T // N_CORES         # 512
JT = SHARD // 128            # 4

f32 = mybir.dt.float32
bf16 = mybir.dt.bfloat16
i32 = mybir.dt.int32
i16 = mybir.dt.int16
u32 = mybir.dt.uint32

_kernel_cache = {}


def _build(has_br: bool, has_b2: bool, reps: int = 1):
    nc = bacc.Bacc("TRN2", target_bir_lowering=False, debug=False,
                   num_devices=N_CORES)
    xbf = nc.dram_tensor("xbf", [T, D], bf16, kind="ExternalInput")
    xsliceT = nc.dram_tensor("xsliceT", [128, 8 * SHARD], f32,
                             kind="ExternalInput")
    w1a = nc.dram_tensor("w1a", [32, 128, 8 * 128], bf16,
                         kind="ExternalInput")
    b1s = nc.dram_tensor("b1s", [H], f32, kind="ExternalInput")
    w2a = nc.dram_tensor("w2a", [2, 32, 128, 512], bf16,
                         kind="ExternalInput")
    b2s = nc.dram_tensor("b2s", [D], f32, kind="ExternalInput")
    wr = nc.dram_tensor("wr", [D, E], f32, kind="ExternalInput")
    br = nc.dram_tensor("br", [E], f32, kind="ExternalInput")
    identc = nc.dram_tensor("identc", [128, 128], f32, kind="ExternalInput")
    iota4 = nc.dram_tensor("iota4", [128, JT], f32, kind="ExternalInput")
    slotio = nc.dram_tensor("slotio", [16, 256], f32, kind="ExternalInput")
    onesrow = nc.dram_tensor("onesrow", [1, 128], f32, kind="ExternalInput")
    repm = nc.dram_tensor("repm", [16, 128], f32, kind="ExternalInput")
    out0 = nc.dram_tensor("out0", [SHARD, 512], bf16, kind="ExternalOutput")
    out1 = nc.dram_tensor("out1", [SHARD, 512], bf16, kind="ExternalOutput")
    outs = [out0, out1]

    with tile.TileContext(nc) as tc:
        with tc.tile_pool(name="persist", bufs=1) as persist, \
             tc.tile_pool(name="dram", bufs=1, space="DRAM") as dram:

            ident = persist.tile([128, 128], f32)
            nc.sync.dma_start(ident[:], identc[:])
            wr_sb = persist.tile([128, 8, E], f32)
            nc.sync.dma_start(wr_sb[:], wr[:].rearrange("(o p) e -> p o e", p=128))
            b1_sb = persist.tile([128, 32], f32)
            nc.sync.dma_start(b1_sb[:], b1s[:].rearrange("(o p) -> p o", p=128))
            ones_sb = persist.tile([1, 128], f32)
            nc.sync.dma_start(ones_sb[:], onesrow[:])
            iota_sb = persist.tile([128, JT], f32)
            nc.sync.dma_start(iota_sb[:], iota4[:])
            slot_sb = persist.tile([16, 256], f32)
            nc.sync.dma_start(slot_sb[:], slotio[:])
            repm_sb = persist.tile([16, 128], f32)
            nc.sync.dma_start(repm_sb[:], repm[:])
            if has_br:
                br_row = persist.tile([1, E], f32)
                nc.sync.dma_start(br_row[:], br[None, :])

            lib_sg = nc.gpsimd.load_library(library_config.sparse_gather)

            zeros4 = persist.tile([128, 4, 512], bf16)
            nc.vector.memset(zeros4[:], 0.0)

            partial = [dram.tile([T, 512], bf16, name=f"partial{dn}")
                       for dn in range(2)]
            rs_out = [dram.tile([SHARD, 512], bf16, name=f"rs{dn}")
                      for dn in range(2)]
            agin = dram.tile([E * 2 * SHARD], f32)
            agout = dram.tile([2 * T], f32)

            xcT = persist.tile([128, NCH, 8, CK], bf16)
            hT = persist.tile([128, 32, MPAD], bf16)
            outall = persist.tile([128, 2, MPAD // 128, 512], bf16)
            idx16 = persist.tile([128, NIDX], i16)
            w128 = persist.tile([128, MPAD // 128], f32)
            w2sb1 = persist.tile([128, 32, 512], bf16)

            # ---------- phase 1: local x-slice transpose + router ----------
            with tc.tile_pool(name="p1", bufs=2) as p1, \
                 tc.tile_pool(name="p1ps_s", bufs=1, space="PSUM") as p1ps_s:
                xT = p1.tile([128, 8, SHARD], f32, tag="xT")
                nc.sync.dma_start(
                    xT[:].rearrange("p a b -> p (a b)"), xsliceT[:])
                # router: psum_l[tok, jt, e] accumulated over dk
                psum_l = p1ps_s.tile([128, JT, E], f32, tag="psl")
                for jt in range(JT):
                    for dk in range(8):
                        nc.tensor.matmul(
                            psum_l[:, jt, :],
                            xT[:, dk, jt * 128:(jt + 1) * 128],
                            wr_sb[:, dk, :],
                            start=(dk == 0), stop=(dk == 7))
                logits_sb = p1.tile([128, JT, E], f32, tag="logits")
                nc.vector.tensor_copy(logits_sb[:], psum_l[:])
                if has_br:
                    brb_ps = p1ps_s.tile([128, E], f32, tag="brb")
                    nc.tensor.matmul(brb_ps[:], ones_sb[:, :], br_row[:],
                                     start=True, stop=True)
                    nc.vector.tensor_tensor(
                        logits_sb[:], logits_sb[:],
                        brb_ps[:, None, :].to_broadcast([128, JT, E]),
                        mybir.AluOpType.add)

                # ---------- phase 2: top-2 softmax + encode ----------
                maxes = p1.tile([128, JT, 8], f32, tag="maxes")
                for jt in range(JT):
                    nc.vector.max(maxes[:, jt, :], logits_sb[:, jt, :])
                dif = p1.tile([128, JT, E], f32, tag="dif")
                nc.vector.tensor_tensor(
                    dif[:], logits_sb[:],
                    maxes[:, :, 0:1].to_broadcast([128, JT, E]),
                    mybir.AluOpType.subtract)
                ex = p1.tile([128, JT, E], f32, tag="ex")
                nc.scalar.activation(ex[:], dif[:],
                                     mybir.ActivationFunctionType.Exp)
                keep = p1.tile([128, JT, E], f32, tag="keep")
                nc.vector.tensor_tensor(
                    keep[:], logits_sb[:],
                    maxes[:, :, 1:2].to_broadcast([128, JT, E]),
                    mybir.AluOpType.is_ge)
                ek = p1.tile([128, JT, E], f32, tag="ek")
                nc.vector.tensor_tensor(ek[:], ex[:], keep[:],
                                        mybir.AluOpType.mult)
                ssum = p1.tile([128, JT], f32, tag="ssum")
                nc.vector.tensor_reduce(ssum[:], ek[:], mybir.AxisListType.X,
                                        mybir.AluOpType.add)
                rs_t = p1.tile([128, JT], f32, tag="rs_t")
                nc.vector.reciprocal(rs_t[:], ssum[:])
                wgt = p1.tile([128, JT, E], f32, tag="wgt")
                nc.vector.tensor_tensor(
                    wgt[:], ek[:],
                    rs_t[:, :, None].to_broadcast([128, JT, E]),
                    mybir.AluOpType.mult)

                # encode ALL experts: vsel_e = keep_e ? tok : -1,
                # vw_e = keep_e ? w_e : -1; col layout (e, k, j)
                vboth = p1.tile([128, E, 2, JT], f32, tag="vboth")
                nc.vector.tensor_tensor(
                    vboth[:, :, 0, :],
                    keep[:].rearrange("p j e -> p e j"),
                    iota_sb[:, None, :].to_broadcast([128, E, JT]),
                    mybir.AluOpType.mult)
                nc.vector.tensor_tensor(
                    vboth[:, :, 1, :],
                    wgt[:].rearrange("p j e -> p e j"),
                    keep[:].rearrange("p j e -> p e j"),
                    mybir.AluOpType.add)
                vb_flat = vboth[:].rearrange("p e k j -> p (e k j)")
                nc.vector.tensor_scalar(vb_flat, vb_flat, -1.0, None,
                                        op0=mybir.AluOpType.add)

                # fold [128, 64] -> [16, 512] with PE transposes
                ps64 = p1ps_s.tile([64, 128], f32, tag="ps64")
                nc.tensor.transpose(ps64[:], vb_flat, ident[:])
                sb64 = p1.tile([64, 128], f32, tag="sb64")
                nc.vector.tensor_copy(sb64[:], ps64[:])
                vag = p1.tile([16, E, 2, JT, 8], f32, tag="vag")
                for u in range(8):
                    ps16 = p1ps_s.tile([16, 64], f32, tag=f"ps16_{u % 2}",
                                       name=f"ps16_{u}")
                    nc.tensor.transpose(ps16[:],
                                        sb64[:, u * 16:(u + 1) * 16],
                                        ident[:64, :64])
                    nc.vector.tensor_copy(
                        vag[:, :, :, :, u],
                        ps16[:].rearrange("p (e k j) -> p e k j", e=E, k=2))
                nc.sync.dma_start(
                    agin[:].rearrange("(e p c) -> p e c", e=E, p=16),
                    vag[:].rearrange("p e k j u -> p e (k j u)"))

            # ---------- phase 3: AllToAll + compaction ----------
            nc.gpsimd.collective_compute(
                "AllToAll",
                mybir.AluOpType.bypass,
                replica_groups=[list(range(N_CORES))],
                ins=[agin[:].opt()],
                outs=[agout[:].opt()],
            )

            # W1 prefetch (2 hm blocks per DMA, issued early on the scalar
            # queue; slot rotation lets loads 7.. stream during mm1)
            _p5cm = tc.tile_pool(name="p5", bufs=6)
            p5 = _p5cm.__enter__()
            w1d = []
            for h in range(16):
                wt = p5.tile([128, 2, 8, 128], bf16, tag="w1bf",
                             name=f"w1bf_{h}")
                nc.scalar.dma_start(
                    wt[:].rearrange("p a b c -> p a (b c)"),
                    w1a[2 * h:2 * h + 2].rearrange("a p b -> p a b"))
                w1d.append(wt)

            nfr = nc.alloc_register(mybir.EngineType.Pool, "nfr")
            cregs = [nc.alloc_register(mybir.EngineType.Pool, f"cr{k}")
                     for k in range(NCH)]

            with tc.tile_pool(name="p3", bufs=1) as p3, \
                 tc.tile_pool(name="p3ps", bufs=1, space="PSUM") as p3ps:
                v16b = p3.tile([16, 2, 8, 32], f32)
                nc.sync.dma_start(
                    v16b[:],
                    agout[:].rearrange("(s p k c) -> p k s c",
                                       s=8, p=16, k=2))
                sg_idx = p3.tile([16, 256], f32)
                sg_w = persist.tile([16, 256], f32)
                nfound = p3.tile([1, 1], u32)
                nfound2 = p3.tile([1, 1], u32)
                sg1 = nc.gpsimd.sparse_gather(
                    sg_idx[:],
                    v16b[:, 0].rearrange("p s c -> p (s c)"),
                    num_found=nfound[:])
                bass._add_dep_helper(sg1.ins, lib_sg.ins, False,
                                     "sparse lib preload")
                nc.gpsimd.sparse_gather(
                    sg_w[:],
                    v16b[:, 1].rearrange("p s c -> p (s c)"),
                    num_found=nfound2[:])
                # per-chunk valid counts into Pool registers
                nc.gpsimd.reg_load(nfr, nfound[:1, :1])
                for k in range(NCH):
                    if k == 0:
                        nc.gpsimd.reg_alu(cregs[0], nfr, CK,
                                          mybir.AluOpType.min)
                    else:
                        nc.gpsimd.reg_alu(cregs[k], nfr, CK * k,
                                          mybir.AluOpType.subtract)
                        nc.gpsimd.reg_alu(cregs[k], cregs[k], CK,
                                          mybir.AluOpType.min)
                lib_mlp = nc.gpsimd.load_library(library_config.mlp)

                # broadcast num_found to 16 partitions via a tiny matmul
                nf_f = p3.tile([1, 1], f32)
                nc.vector.tensor_copy(nf_f[:], nfound[:])
                nf_ps = p3ps.tile([16, 1], f32, tag="nf_ps")
                nc.tensor.matmul(nf_ps[:], ones_sb[:, :16], nf_f[:],
                                 start=True, stop=True)
                nf_b = p3.tile([16, 1], f32)
                nc.vector.tensor_copy(nf_b[:], nf_ps[:])
                valid = p3.tile([16, NIDX], i32)
                nc.vector.tensor_tensor(valid[:], slot_sb[:, :NIDX],
                                        nf_b[:].to_broadcast([16, NIDX]),
                                        mybir.AluOpType.is_lt)
                # clean pads to -1, then replicate to the 8 Q7 core groups
                # via one PE matmul and cast to int16
                idxm = p3.tile([16, NIDX], f32)
                nc.vector.memset(idxm[:], -1.0)
                nc.vector.copy_predicated(idxm[:], valid[:],
                                          sg_idx[:, :NIDX])
                idx_ps = p3ps.tile([128, NIDX], f32, tag="idx_ps")
                nc.tensor.matmul(idx_ps[:], repm_sb[:], idxm[:],
                                 start=True, stop=True)
                nc.vector.tensor_copy(idx16[:], idx_ps[:])

                # weight table spread [16, 72] -> [128, 9] (pad slots are
                # never scattered, so no cleanup needed)
                wv = sg_w[:].rearrange("p (c u) -> p c u", u=8)
                for u in range(8):
                    nc.sync.dma_start(w128[u * 16:(u + 1) * 16, :],
                                      wv[:, :MPAD // 128, u])

                # ---------- phase 4: gather selected tokens (transposed) ----
                for k in range(NCH):
                    g = nc.gpsimd.dma_gather(
                        xcT[:, k], xbf[:],
                        idx16[:, 24 * k:24 * (k + 1)], CK, cregs[k], D,
                        transpose=True)
                    if k == 0:
                        bass._add_dep_helper(g.ins, lib_mlp.ins, False,
                                             "mlp lib preload")

            # zero-fill the RS partial buffers + preload W2 dn=1 (batched;
            # program order places these after the critical gather issue)
            for dn in range(2):
                for j in range(T // 512):
                    nc.sync.dma_start(
                        partial[dn][j * 512:(j + 1) * 512, :]
                        .rearrange("(a p) c -> p a c", p=128), zeros4[:])
            for q in range(8):
                nc.sync.dma_start(
                    w2sb1[:, 4 * q:4 * (q + 1), :],
                    w2a[1, 4 * q:4 * (q + 1)].rearrange("a p b -> p a b"))

            # ---------- phase 5: mm1 (hT = gelu(W1^T xc^T + b1)) ----------
            with tc.tile_pool(name="p5ps", bufs=3, space="PSUM") as p5ps:
                for hm in range(32):
                    w1bf = w1d[hm // 2]
                    for k in range(NCH):
                        psum = p5ps.tile([128, CK], f32, tag="mm1",
                                         name=f"mm1ps_{k}_{hm}")
                        for dk in range(8):
                            nc.tensor.matmul(
                                psum[:], w1bf[:, hm % 2, dk, :],
                                xcT[:, k, dk, :],
                                start=(dk == 0), stop=(dk == 7))
                        nc.scalar.activation(
                            hT[:, hm, k * CK:(k + 1) * CK], psum[:],
                            mybir.ActivationFunctionType.Gelu,
                            bias=b1_sb[:, hm:hm + 1])
            _p5cm.__exit__(None, None, None)

            # ---------- phase 6: mm2 + weight + scatter-add + RS ----------
            with tc.tile_pool(name="p6", bufs=8) as p6, \
                 tc.tile_pool(name="p6o", bufs=2) as p6o, \
                 tc.tile_pool(name="p6ps", bufs=1, space="PSUM") as p6ps:
                for dn in range(2):
                    for g in range(NCH):
                        psum_o = {}
                        for tb in range(3 * g, 3 * g + 3):
                            psum_o[tb] = p6ps.tile(
                                [128, 512], f32, tag=f"mm2_{tb - 3 * g}",
                                name=f"mm2ps_{dn}_{tb}")
                        for hk in range(32):
                            if dn == 0:
                                if hk % 4 == 0:
                                    w2bf = p6.tile([128, 4, 512], bf16,
                                                   tag="w2bf",
                                                   name=f"w2bf_{g}_{hk}")
                                    nc.scalar.dma_start(
                                        w2bf[:],
                                        w2a[0, hk:hk + 4]
                                        .rearrange("a p b -> p a b"))
                                w2op = w2bf[:, hk % 4, :]
                            else:
                                w2op = w2sb1[:, hk, :]
                            for tb in range(3 * g, 3 * g + 3):
                                nc.tensor.matmul(
                                    psum_o[tb],
                                    hT[:, hk, tb * 128:(tb + 1) * 128],
                                    w2op,
                                    start=(hk == 0), stop=(hk == 31))
                        for tb in range(3 * g, 3 * g + 3):
                            if has_b2:
                                outf = p6o.tile([128, 512], f32, tag="outf")
                                nc.vector.tensor_scalar_mul(
                                    outf[:], psum_o[tb], w128[:, tb:tb + 1])
                                b2sb = p6o.tile([1, 512], f32, tag="b2sb")
                                nc.sync.dma_start(
                                    b2sb[:], b2s[None,
                                                 dn * 512:(dn + 1) * 512])
                                b2ps = p6ps.tile([128, 512], f32, tag="b2ps")
                                nc.tensor.matmul(
                                    b2ps[:], ones_sb[:, :], b2sb[:],
                                    start=True, stop=True)
                                b2w = p6o.tile([128, 512], f32, tag="b2w")
                                nc.vector.tensor_scalar_mul(
                                    b2w[:], b2ps[:], w128[:, tb:tb + 1])
                                nc.vector.tensor_tensor(
                                    outf[:], outf[:], b2w[:],
                                    mybir.AluOpType.add)
                                nc.vector.tensor_copy(
                                    outall[:, dn, tb, :], outf[:])
                            else:
                                nc.vector.tensor_scalar_mul(
                                    outall[:, dn, tb, :], psum_o[tb],
                                    w128[:, tb:tb + 1])
                        nc.gpsimd.dma_scatter_add(
                            partial[dn][:],
                            outall[:, dn, 3 * g:3 * g + 3, :],
                            idx16[:, 24 * g:24 * (g + 1)],
                            CK, cregs[g], 512)
                    nc.gpsimd.collective_compute(
                        "ReduceScatter",
                        mybir.AluOpType.add,
                        replica_groups=[list(range(N_CORES))],
                        ins=[partial[dn][:].opt()],
                        outs=[rs_out[dn][:].opt()],
                    )
                    nc.sync.dma_start(outs[dn][:], rs_out[dn][:])

    nc.compile()
    return nc


def _get_kernel(has_br: bool, has_b2: bool, reps: int = 1):
    key = (has_br, has_b2, reps)
    if key not in _kernel_cache:
        _kernel_cache[key] = _build(has_br, has_b2, reps)
    return _kernel_cache[key]


def _const_inputs():
    identc = np.eye(128, dtype=np.float32)
    slotio = (np.arange(256)[None, :] * 16
              + np.arange(16)[:, None]).astype(np.float32)
    onesrow = np.ones((1, 128), np.float32)
    repm = np.tile(np.eye(16, dtype=np.float32), (1, 8))
    return identc, slotio, onesrow, repm


def make_in_maps(x, W1, b1, W2, b2, Wr, br):
    xf = np.ascontiguousarray(np.asarray(x, np.float32).reshape(T, D))
    xbf = np.ascontiguousarray(xf.astype(ml_dtypes.bfloat16))
    W1 = np.asarray(W1, dtype=np.float32).astype(ml_dtypes.bfloat16)
    b1 = np.asarray(b1, dtype=np.float32)
    W2 = np.asarray(W2, dtype=np.float32).astype(ml_dtypes.bfloat16)
    b2 = np.asarray(b2, dtype=np.float32)
    Wr = np.ascontiguousarray(np.asarray(Wr, dtype=np.float32))
    br = np.ascontiguousarray(np.asarray(br, dtype=np.float32))
    identc, slotio, onesrow, repm = _const_inputs()
    in_maps = []
    for r in range(N_CORES):
        iota = (512 * r + np.arange(JT)[None, :] * 128
                + np.arange(128)[:, None] + 1.0).astype(np.float32)
        xs = xf[512 * r:512 * (r + 1)]                       # [512, 1024]
        xsT = np.ascontiguousarray(
            xs.T.reshape(8, 128, SHARD).transpose(1, 0, 2)
            .reshape(128, 8 * SHARD))
        w1r = np.ascontiguousarray(
            W1[r].reshape(8, 128, 32, 128).transpose(2, 1, 0, 3)
            .reshape(32, 128, 8 * 128))
        w2r = np.ascontiguousarray(
            W2[r].reshape(32, 128, 2, 512).transpose(2, 0, 1, 3))
        in_maps.append({
            "xbf": xbf,
            "xsliceT": xsT,
            "w1a": w1r,
            "b1s": np.ascontiguousarray(b1[r]),
            "w2a": w2r,
            "b2s": np.ascontiguousarray(b2[r]),
            "wr": Wr,
            "br": br,
            "identc": identc,
            "iota4": iota,
            "slotio": slotio,
            "onesrow": onesrow,
            "repm": repm,
        })
    return in_maps


def kernel(x, W1, b1, W2, b2, Wr, br):
    x = np.asarray(x, dtype=np.float32)
    B, S, _ = x.shape
    has_br = bool(np.any(np.asarray(br)))
    has_b2 = bool(np.any(np.asarray(b2)))
    nc = _get_kernel(has_br, has_b2)
    in_maps = make_in_maps(x, W1, b1, W2, b2, Wr, br)
    res = bass_utils.run_bass_kernel_spmd(
        nc, in_maps, core_ids=list(range(N_CORES)))
    out = np.concatenate(
        [np.concatenate([np.asarray(res.results[r]["out0"]),
                         np.asarray(res.results[r]["out1"])], axis=1)
         for r in range(N_CORES)], axis=0)
    return out.astype(np.float32).reshape(B, S, D)
